# revision 61
# baseline (speedup 1.0000x reference)
"""Causal attention kernel for Trainium2 (8 NeuronCores).

Problem: B=2, H=16, S=2048, D=64 causal attention with a softmax whose
global-max subtraction cancels mathematically (softmax is shift-invariant),
so an unshifted softmax is numerically equivalent.

Sharding: the 32 (b,h) heads are split 4-per-core across 8 cores
(head-parallel, no communication). Q and K are pre-transposed on the host to
[head, D, S] bf16 so the on-chip [d, s] layout (contraction dim d on
partitions) loads with contiguous DMA.

Per-core kernel (per head, scores computed in S^T = [k, q] layout):
  - QK: S^T[k_chunk, q_block] = matmul(lhsT=Kt chunk [64,128],
    rhs=Qt block [64,512]) in bf16, grouped two chunks per PSUM tile
    ([128, 1024], 2 banks, 3-buffer rotation).
  - exp(0.125 * S^T): split across TWO engines, greedy-balanced per unit
    (GpSimd cannot read PSUM, so it cannot help here):
      * ACT: exact table exp (scale folded in), bf16 out.
      * DVE: Schraudolph bit-trick exp in ONE tensor_scalar:
        e_bf16 = bitcast( int16( s*(0.125*log2e*128) + (16256 - 7.37) ) ).
        The -7.37 centers the piecewise-linear mantissa error so its mean
        ratio vs true exp is 1.0000 (softmax-normalization-safe when mixed
        with exact-ACT columns); residual ~1.7% rms noise on those columns.
    Diagonal 128-blocks of e are triangle-zeroed AFTER the exp by
    affine_select on GpSimd; the packed unit keeps its three diagonal
    blocks contiguous so ONE affine_select (flat iota pattern [[0,3],[1,128]])
    zeros all three.
  - PV (restructured vs the usual form): e is the STATIONARY operand.
    For each 128-wide q sub-block j and k-chunk ki:
      po[q 128, j, 65] += matmul(lhsT=e[k 128, q 128], rhs=Vplus[k 128, 65])
    where Vplus carries a ones column so column 64 accumulates softmax row
    sums. Only 65 columns stream per (q,k) block pair (the cost model charges
    matmuls by output free size; ldweights is free), and the output lands
    directly in [q, d] layout - no PE transposes, no PSUM->SBUF copy.
    start=True clears has_written for the WHOLE PSUM bank (hw-verified), so
    exactly one start (block's first PV) and one stop (its last) are issued;
    every other region first-touch overwrites via has_written=0.
  - Epilogue per q-block: one reciprocal [128,4] of the sum columns, one
    broadcast multiply to f32 SBUF, one DMA per 512 rows.

Scheduling: small blocks (qb0/qb1) lack the PE work to hide exp latency in
the 3-deep PSUM rotation, so each head interleaves a big and a small block
(the two po accumulators exactly fill the 2-buffer po pool), staggered so
the four GpSimd triangle-zeros of a pair don't queue up behind each other;
within a block the diagonal and packed units go first so their zeros
complete several unit-slots before their PVs pop. PV matmuls and epilogues
go through a deferred-action FIFO pumped BEFORE each unit's QKs (stalled QKs
then have ready PV work queued ahead); head 0 loads in chunks on the SP+ACT
HWDGE queues (qt first) and runs smallest-blocks-first to match; later heads
prefetch whole tensors on the SP queue (loads never data-wait; GpSimd must
stay clear for the zeros); warmup matmuls on framework const tiles start the
clock ramp right after the init barrier; the last pair front-loads its small
block so only one store is exposed in the drain.
"""

import numpy as np

B, H, S, D = 2, 16, 2048, 64
N_CORES = 8
HPC = (B * H) // N_CORES  # heads per core = 4
QB = 512  # q-block width
KB = 128  # k-chunk width
NQB = S // QB  # 4
NKB = S // KB  # 16

# Schraudolph exp constants: e = bitcast_bf16(int16(s*C1 + C2)) ~ exp(s/8)
C1 = 0.125 * 1.4426950408889634 * 128.0
C2 = 16256.0 - 7.37

# engine-busy cost estimates (ns) for the exp balancer
ACT_NS = 0.8333
DVE_NS = 1.125  # 1.0417 actual; slight ACT bias measures faster + more accurate
POOL_NS = 1.389

_CACHED = {}


def _build_nc():
    import concourse.bacc as bacc
    import concourse.mybir as mybir
    from concourse.tile import TileContext

    f32 = mybir.dt.float32
    bf16 = mybir.dt.bfloat16
    i16 = mybir.dt.int16
    EXP = mybir.ActivationFunctionType.Exp
    MULT = mybir.AluOpType.mult
    ADD = mybir.AluOpType.add

    nc = bacc.Bacc()
    Qd = nc.declare_dram_parameter("Qt", [HPC, D, S], bf16, isOutput=False)
    Kd = nc.declare_dram_parameter("Kt", [HPC, D, S], bf16, isOutput=False)
    Vd = nc.declare_dram_parameter("V", [HPC, S, D], bf16, isOutput=False)
    Od = nc.declare_dram_parameter("out", [HPC, S, D], f32, isOutput=True)

    with TileContext(nc) as tc:
        with (
            tc.tile_pool(name="consts", bufs=1) as cpool,
            tc.tile_pool(name="qt", bufs=2) as qt_pool,
            tc.tile_pool(name="kt", bufs=2) as kt_pool,
            tc.tile_pool(name="vp", bufs=2) as v_pool,
            tc.tile_pool(name="e", bufs=8) as e_pool,
            tc.tile_pool(name="oo", bufs=3) as oo_pool,
            tc.tile_pool(name="r", bufs=3) as r_pool,
            tc.tile_pool(name="ps", bufs=3, space="PSUM") as ps_pool,
            tc.tile_pool(name="po", bufs=2, space="PSUM") as po_pool,
        ):
            # PE warmup: dummy matmuls so the clock ramp starts at t=0.
            # zero-fills go on GpSimd (fastest engine to first SBUF write;
            # ACT is blocked by its activation-table load until ~2.1us).
            # Tiny 64-wide matmuls start earlier (small memsets) and quantize
            # the warmup finely, so real QKs start the moment data lands.
            wa = nc.const_aps.tensor(1.0, [64, 1], bf16)
            wb = nc.const_aps.tensor(1.0, [64, 64], bf16)
            wp = ps_pool.tile([KB, 2 * QB], f32, tag="ps")
            for _ in range(52):
                nc.tensor.matmul(wp[0:1, 0:64], lhsT=wa, rhs=wb, start=True, stop=True)

            # lower-triangle keep-mask (1 where q >= k) for the final diag
            # unit, whose masking runs as a DVE multiply during the drain
            mask_t = cpool.tile([128, KB], bf16)
            nc.gpsimd.memset(mask_t[:], 1.0)
            nc.gpsimd.affine_select(
                out=mask_t[:],
                in_=mask_t[:],
                compare_op=mybir.AluOpType.is_ge,
                fill=0.0,
                base=0,
                pattern=[[1, KB]],
                channel_multiplier=-1,
            )

            # exp engine balancer state: projected busy-ns per engine
            busy = {"act": 0.0, "dve": 0.0, "pool": 0.0}

            def _exp_one(eng, e_ap_bf16, ps_ap):
                if eng == "act":
                    nc.scalar.activation(e_ap_bf16, ps_ap, EXP, scale=0.125)
                else:
                    nc.vector.tensor_scalar(
                        e_ap_bf16.bitcast(i16), ps_ap, C1, C2, MULT, ADD
                    )

            def do_exp(e_ap_bf16, ps_ap, cols, force=None):
                # GPSIMD cannot read PSUM, so only ACT and DVE split the exp
                ca = cols * ACT_NS + 185.0
                cd = cols * DVE_NS + 125.0
                fin = {
                    "act": busy["act"] + ca,
                    "dve": busy["dve"] + cd,
                }
                eng = force or min(fin, key=fin.get)
                busy[eng] = fin[eng]
                _exp_one(eng, e_ap_bf16, ps_ap)

            def causal_zero(e_blk):
                # zero e[i, j] for j < i (future positions) on GpSimd
                busy["pool"] += 273.0
                nc.gpsimd.affine_select(
                    out=e_blk,
                    in_=e_blk,
                    compare_op=mybir.AluOpType.is_ge,
                    fill=0.0,
                    base=0,
                    pattern=[[1, KB]],
                    channel_multiplier=-1,
                )

            def load_head(h):
                qt = qt_pool.tile([D, S], bf16, tag="qt")
                kt = kt_pool.tile([D, S], bf16, tag="kt")
                vp = v_pool.tile([128, NKB, D + 1], bf16, tag="vp")
                nc.gpsimd.memset(vp[:, :, D], 1.0)
                vr = Vd[h].rearrange("(c p) d -> p c d", p=128)
                if h == 0:
                    # prologue: nothing to overlap with - fast HWDGE queues.
                    # qt part 1 is emitted first: the shared HWDGE device
                    # serializes descriptor generation in emission order and
                    # the first QK needs qt+kt part 1 together.
                    parts = [(0, 512), (512, 1024), (1024, 2048)]
                    for pi, (a, b) in enumerate(parts):
                        sl = slice(a, b)
                        nc.scalar.dma_start(out=qt[:, sl], in_=Qd[h, :, sl])
                        nc.sync.dma_start(out=kt[:, sl], in_=Kd[h, :, sl])
                        csl = slice(a // KB, b // KB)
                        nc.sync.dma_start(out=vp[:, csl, 0:D], in_=vr[:, csl, :])
                else:
                    # steady state: prefetch loads ride the SP HWDGE queue.
                    # Loads never data-wait (DRAM source is ready), so they
                    # cannot block the queue; GpSimd must stay clear for the
                    # causal zeros, whose latency gates the diagonal PVs.
                    nc.sync.dma_start(out=kt[:], in_=Kd[h])
                    nc.sync.dma_start(out=qt[:], in_=Qd[h])
                    nc.sync.dma_start(out=vp[:, :, 0:D], in_=vr[:])
                return qt, kt, vp

            # deferred-action FIFO: PV matmul batches and epilogues trail the
            # QK/exp stream so nothing data-waits at the PE queue head
            actions = []

            _PUMP_L = 3
            _PUMP_D = 6
            # taper the pump depth near the end of emission so the final
            # epilogues don't bunch up in the drain flush
            sched = {"remaining": HPC * 20}

            def pump(limit=None, depth=None):
                limit = _PUMP_L if limit is None else limit
                depth = min(
                    _PUMP_D if depth is None else depth,
                    max(1, sched["remaining"]),
                )
                n = 0
                while actions and len(actions) > depth and n < limit:
                    actions.pop(0)()
                    n += 1

            def make_pv(po, vp, qb, pvs):
                # pvs: list of (ki, j, e_slice_ap, start, stop)
                # NOTE: start=True clears has_written for the WHOLE PSUM bank
                # (hw-verified), so exactly one start (the block's first PV)
                # and one stop (its last PV) are allowed per po bank; every
                # other mm relies on per-element has_written=0 -> overwrite
                # for its own first touch of a region.
                def act():
                    for ki, j, e_ap, st, sp in pvs:
                        nc.tensor.matmul(
                            po[:, j, :],
                            lhsT=e_ap,
                            rhs=vp[:, ki, :],
                            start=st,
                            stop=sp,
                            skip_group_check=True,
                        )

                return act

            def make_epilogue(h, qb, po):
                def act():
                    q0 = qb * QB
                    oo = oo_pool.tile([128, 4, D], f32, tag="oo")
                    busy["dve"] += 520.0
                    r = r_pool.tile([128, 4], f32, tag="r")
                    nc.vector.reciprocal(r[:], po[:, :, D])
                    nc.vector.tensor_mul(
                        oo[:],
                        po[:, :, 0:D],
                        r[:].unsqueeze(2).broadcast_to([128, 4, D]),
                    )
                    nc.sync.dma_start(
                        out=Od[h, q0 : q0 + QB, :].rearrange(
                            "(j p) d -> p j d", p=128
                        ),
                        in_=oo[:],
                    )

                return act

            tiles = load_head(0)
            for h in range(HPC):
                qt, kt, vp = tiles

                def emit_unit(h, qb, po, qt, kt, vp, kind, ki0, n, blk, dve_mask=False):
                    # pop deferred PV work first: it lands in the PE stream
                    # BEFORE this unit's QKs, so a QK stalled on the psum
                    # rotation has ready PV work queued ahead of it
                    sched["remaining"] -= 1
                    pump()
                    q0 = qb * QB
                    ps = ps_pool.tile([KB, 2 * QB], f32, tag="ps")
                    e = e_pool.tile([KB, 2 * QB], bf16, tag="e")
                    pvs = []
                    if kind == "grp":
                        for i in range(n):
                            ki = ki0 + i
                            nc.tensor.matmul(
                                ps[:, i * QB : (i + 1) * QB],
                                lhsT=kt[:, ki * KB : (ki + 1) * KB],
                                rhs=qt[:, q0 : q0 + QB],
                                start=True,
                                stop=True,
                            )
                        cols = n * QB
                        do_exp(
                            e[:, 0:cols], ps[:, 0:cols], cols,
                            force="act" if dve_mask else None,
                        )
                        if ki0 + n - 1 == 4 * qb:  # contains the diagonal
                            dg = slice((n - 1) * QB, (n - 1) * QB + KB)
                            if dve_mask:
                                # drain path: mask on DVE right behind its
                                # exp - no GpSimd hop in the final chain
                                busy["dve"] += 200.0
                                nc.vector.tensor_mul(e[:, dg], e[:, dg], mask_t[:])
                            else:
                                causal_zero(e[:, dg])
                        for i in range(n):
                            ki = ki0 + i
                            for j in range(4):
                                pvs.append(
                                    (ki, j, e[:, i * QB + j * KB : i * QB + (j + 1) * KB])
                                )
                    else:
                        # packed partial-diagonal chunks, bank-aligned, with
                        # the three diagonal 128-blocks CONTIGUOUS up front so
                        # one affine_select zeros all of them:
                        #   [0:128]   diag of off=128 (q 128-255)
                        #   [128:256] diag of off=256 (q 256-383)
                        #   [256:384] diag of off=384 (q 384-511)
                        #   [384:512] rest of off=256 (q 384-511)
                        #   [512:768] rest of off=128 (q 256-511)
                        mms = [
                            (4 * qb + 1, KB, 0, KB),
                            (4 * qb + 2, 2 * KB, KB, KB),
                            (4 * qb + 3, 3 * KB, 2 * KB, KB),
                            (4 * qb + 2, 3 * KB, 3 * KB, KB),
                            (4 * qb + 1, 2 * KB, 4 * KB, 2 * KB),
                        ]
                        for kk, qoff, base, w in mms:
                            nc.tensor.matmul(
                                ps[:, base : base + w],
                                lhsT=kt[:, kk * KB : (kk + 1) * KB],
                                rhs=qt[:, q0 + qoff : q0 + qoff + w],
                                start=True,
                                stop=True,
                            )
                        do_exp(e[:, 0:768], ps[:, 0:768], 768)
                        busy["pool"] += 628.0
                        nc.gpsimd.affine_select(
                            out=e[:, 0:384],
                            in_=e[:, 0:384],
                            compare_op=mybir.AluOpType.is_ge,
                            fill=0.0,
                            base=0,
                            pattern=[[0, 3], [1, KB]],
                            channel_multiplier=-1,
                        )
                        for kk, qoff, base, w in mms:
                            for jj in range(w // KB):
                                j = (qoff + jj * KB) // KB
                                pvs.append(
                                    (kk, j, e[:, base + jj * KB : base + (jj + 1) * KB])
                                )
                    # the bank's single start rides the block's first-emitted
                    # PV (accumulation order across chunks is free); its
                    # single stop rides the last one
                    flagged = []
                    for ki, j, ap in pvs:
                        st = not blk["started"]
                        blk["started"] = True
                        flagged.append((ki, j, ap, st, False))
                    if blk["units_left"] == 1 and flagged:
                        ki_l, j_l, ap_l, st_l, _ = flagged[-1]
                        flagged[-1] = (ki_l, j_l, ap_l, st_l, True)
                    blk["units_left"] -= 1
                    actions.append(make_pv(po, vp, qb, flagged))

                def block_thunks(h, qb, qt, kt, vp, tail=False, chunk1=False):
                    # returns unit-emission thunks; the block's epilogue is
                    # appended by the caller after the last thunk runs.
                    # The diagonal and packed units go FIRST: their GpSimd
                    # triangle-zeros then complete several unit-slots before
                    # their PVs pop, keeping the zeros off the PE critical
                    # path. The drain block (tail=True) instead ENDS on the
                    # small diag unit, masked on the DVE.
                    po = po_pool.tile([128, NQB, D + 1], f32, tag="po")
                    nfull = 4 * qb + 1
                    grps = []
                    ki0 = 0
                    step = 1 if chunk1 else 2
                    while ki0 < nfull:
                        nn = min(step, nfull - ki0)
                        grps.append(("grp", ki0, nn))
                        ki0 += nn
                    if tail:
                        # drain block: diag+packed first (zeros early), end
                        # on 1-chunk units so the final exp is short
                        head_units = [grps[-1], ("packed", 0, 0)] + grps[:-2]
                        kk0 = grps[-2][1] if len(grps) > 1 else None
                        units = head_units + (
                            [("grp", kk0, 1), ("grp", kk0 + 1, 1)]
                            if kk0 is not None
                            else []
                        )
                    else:
                        units = [grps[-1], ("packed", 0, 0)] + grps[:-1]
                    blk = {"started": False, "units_left": len(units)}
                    return po, [
                        (lambda kind=kind, k0=k0, nn=nn: emit_unit(
                            h, qb, po, qt, kt, vp, kind, k0, nn, blk,
                            dve_mask=(tail and kind == "grp" and k0 + nn - 1 == 4 * qb),
                        ))
                        for kind, k0, nn in units
                    ]

                # Small blocks (qb0/qb1) have too little PE work per unit to
                # hide the exp latency of the 3-deep PSUM rotation, so each
                # head interleaves a big block with a small one; the two po
                # accumulators exactly fill the 2-buffer po pool. Head 0 runs
                # smallest-first to match the incremental part loads; the
                # last head ends on a small packed unit for a fast drain.
                if h == 0:
                    pairs = [(0, 1), (2, 3)]
                elif h == HPC - 1:
                    pairs = [(3, 2), (1, 0)]
                else:
                    pairs = [(3, 0), (2, 1)]
                for pi, (qa, qb_) in enumerate(pairs):
                    last_pair = h == HPC - 1 and pi == len(pairs) - 1
                    poA, TA = block_thunks(h, qa, qt, kt, vp)
                    poB, TB = block_thunks(h, qb_, qt, kt, vp)
                    na, nb = len(TA), len(TB)
                    seq = []
                    if last_pair:
                        # front-load the small block so its epilogue + store
                        # drain while the big block still computes; only the
                        # big block's store is exposed at the end
                        for i in range(max(na, nb)):
                            if i < nb:
                                seq.append(("B", TB[i], i == nb - 1))
                            if i < na:
                                seq.append(("A", TA[i], i == na - 1))
                    else:
                        if True:
                            idx = [("A",0),("A",1),("B",0),("A",2),("B",1),("B",2)]
                            done = set(idx)
                            for w, i in idx:
                                T = TA if w == "A" else TB
                                nn = na if w == "A" else nb
                                if i < nn:
                                    seq.append((w, T[i], i == nn - 1))
                            for i in range(max(na, nb)):
                                for w, T, nn in (("A", TA, na), ("B", TB, nb)):
                                    if i < nn and (w, i) not in done:
                                        seq.append((w, T[i], i == nn - 1))
                        else:
                            for i in range(max(na, nb)):
                                if i < na:
                                    seq.append(("A", TA[i], i == na - 1))
                                if i < nb:
                                    seq.append(("B", TB[i], i == nb - 1))
                    for which, thunk, is_last in seq:
                        thunk()
                        if is_last:
                            qq = qa if which == "A" else qb_
                            pp = poA if which == "A" else poB
                            actions.append(make_epilogue(h, qq, pp))
                            pump()

                    # prefetch the next head midway through this head
                    if pi == 0 and h + 1 < HPC:
                        next_tiles = load_head(h + 1)
                if h + 1 < HPC:
                    tiles = next_tiles  # noqa: F821

            while actions:
                actions.pop(0)()
    nc.finalize()
    return nc


def _get_nc():
    if "nc" not in _CACHED:
        _CACHED["nc"] = _build_nc()
    return _CACHED["nc"]


def kernel(Q, K, V, mask=None, **_ignored):
    import ml_dtypes
    from concourse.bass_utils import run_bass_kernel_spmd

    nc = _get_nc()
    bf16 = ml_dtypes.bfloat16
    Qr = np.ascontiguousarray(
        np.asarray(Q, dtype=np.float32).reshape(B * H, S, D).transpose(0, 2, 1)
    ).astype(bf16)
    Kr = np.ascontiguousarray(
        np.asarray(K, dtype=np.float32).reshape(B * H, S, D).transpose(0, 2, 1)
    ).astype(bf16)
    Vr = np.asarray(V, dtype=np.float32).reshape(B * H, S, D).astype(bf16)
    in_maps = [
        {
            "Qt": Qr[i * HPC : (i + 1) * HPC],
            "Kt": Kr[i * HPC : (i + 1) * HPC],
            "V": Vr[i * HPC : (i + 1) * HPC],
        }
        for i in range(N_CORES)
    ]
    res = run_bass_kernel_spmd(nc, in_maps, core_ids=list(range(N_CORES)))
    out = np.concatenate([res.results[i]["out"] for i in range(N_CORES)], axis=0)
    return out.reshape(B, H, S, D).astype(np.float32)
